# revision 5
# baseline (speedup 1.0000x reference)
"""Llama4 MoE experts (grouped GEMM + SwiGLU) on 8 Trainium2 NeuronCores.

Expert-parallel: core e computes expert e's token block
  Y_e = (silu(X_e @ Wg_e) * (X_e @ Wu_e)) @ Wd_e
with X_e = hidden_states[e*1024:(e+1)*1024]. No collectives needed.

All matmul operands are bf16 (PE full rate, FWL fast weight load); PSUM
accumulation is fp32 and the output is fp32, so end-to-end error stays
~4e-3. The host pre-packs per-core inputs so the device does no casts
and no transposes:
  - xt:  X^T as [8, 128, 4, 1024]  (h on partitions, tokens free)
  - w1:  gate/up interleaved [32, 4, 128, 8, 256] (per d-tile j: 128
         gate cols then 128 up cols, k-batched for 512KB linear DMAs)
  - w2:  Wd as [8, 4, 128, 8, 512] (per 512-wide output col group)
Per-core dataflow:
  MM1: for each d-tile j: accumulate gate/up psum over k (4 PSUM banks
       per group, two groups ping-pong), SwiGLU (Silu on ScalarE + one
       DVE mul) -> act slab bf16 [128, 32, 1024].
  MM2: for each token half/col group: psy[4] accumulate over kd with
       Wd moving; ScalarE evicts to an SBUF row slab, DMA out fp32.
Weights stream once (Wgu) / twice (Wd, once per token half).
"""
from contextlib import ExitStack

import ml_dtypes
import numpy as np

import concourse.bass as bass
import concourse.tile as tile
from concourse import bacc, mybir
from concourse.bass_utils import run_bass_kernel_spmd

P = 128
F32 = mybir.dt.float32
BF16 = mybir.dt.bfloat16
SILU = mybir.ActivationFunctionType.Silu
COPY = mybir.ActivationFunctionType.Copy

E = 8            # experts == cores
T = 1024         # tokens per expert
H = 4096         # hidden
D = 4096         # expert (intermediate) dim

KH = H // P      # 32 contraction tiles for MM1
KD = D // P      # 32 contraction tiles for MM2
NJ = D // P      # 32 gate/up d-tile groups
TC = T // 512    # 2 token chunks (psum free dim 512)
NH = H // 512    # 8 output column groups
TTH = 2          # token halves for MM2
TT = T // TTH // P   # 4 token tiles per half
KB = 8           # k tiles per weight DMA kick

_cached_nc = None


def _build_program():
    nc = bacc.Bacc("TRN2", target_bir_lowering=False, debug=False)
    xt_d = nc.dram_tensor("xt", [KH // 4, P, 4, T], BF16, kind="ExternalInput").ap()
    w1_d = nc.dram_tensor("w1", [NJ, KH // KB, P, KB, 2 * P], BF16,
                          kind="ExternalInput").ap()
    w2_d = nc.dram_tensor("w2", [NH, KD // KB, P, KB, 512], BF16,
                          kind="ExternalInput").ap()
    # y laid out [th, tt, p, h] — linearly identical to [T, H]
    y_d = nc.dram_tensor("y", [TTH, T // TTH // P, P, H], F32,
                         kind="ExternalOutput").ap()

    with tile.TileContext(nc) as tc, ExitStack() as ctx:
        slab = ctx.enter_context(tc.tile_pool(name="slab", bufs=1))
        xt = slab.tile([P, KH, T], BF16, tag="xt")
        act = slab.tile([P, KD, T], BF16, tag="act")

        w1s = ctx.enter_context(tc.tile_pool(name="w1s", bufs=3))
        w2s = ctx.enter_context(tc.tile_pool(name="w2s", bufs=2))
        stmp = ctx.enter_context(tc.tile_pool(name="stmp", bufs=4))
        yout = ctx.enter_context(tc.tile_pool(name="yout", bufs=2))
        ps = ctx.enter_context(tc.tile_pool(name="ps", bufs=8, space="PSUM"))

        # ---- load X^T (8 linear kicks of 1 MiB) ----
        for b in range(KH // 4):
            nc.sync.dma_start(xt[:, b * 4:(b + 1) * 4, :], xt_d[b])

        # ---- MM1 (gate/up) + SwiGLU -> act ----
        for j in range(NJ):
            psg = [ps.tile([P, 512], F32, tag="ps", name="psg") for _ in range(TC)]
            psu = [ps.tile([P, 512], F32, tag="ps", name="psu") for _ in range(TC)]
            for kb in range(KH // KB):
                wc = w1s.tile([P, KB, 2 * P], BF16, name="w1c")
                nc.sync.dma_start(wc[:], w1_d[j, kb])
                for kk in range(KB):
                    k = kb * KB + kk
                    for which, lst in ((0, psg), (1, psu)):
                        for tc_ in range(TC):
                            nc.tensor.matmul(
                                lst[tc_][:],
                                wc[:, kk, which * P:(which + 1) * P],
                                xt[:, k, tc_ * 512:(tc_ + 1) * 512],
                                start=(k == 0), stop=(k == KH - 1))
            for tc_ in range(TC):
                st = stmp.tile([P, 512], F32, name="st")
                nc.scalar.activation(st[:], psg[tc_][:], SILU)
                nc.vector.tensor_mul(
                    act[:, j, tc_ * 512:(tc_ + 1) * 512], psu[tc_][:], st[:])

        # ---- MM2 (down projection) ----
        for th in range(TTH):
            t0 = th * (T // TTH)
            for nh in range(NH):
                psy = [ps.tile([P, 512], F32, tag="ps", name="psy")
                       for _ in range(TT)]
                for kb in range(KD // KB):
                    wc = w2s.tile([P, KB, 512], BF16, name="w2c")
                    nc.scalar.dma_start(wc[:], w2_d[nh, kb])
                    for kk in range(KB):
                        kd = kb * KB + kk
                        for mt in range(TT):
                            nc.tensor.matmul(
                                psy[mt][:],
                                act[:, kd, t0 + mt * P:t0 + (mt + 1) * P],
                                wc[:, kk, :],
                                start=(kd == 0), stop=(kd == KD - 1))
                yo = yout.tile([P, TT, 512], F32, name="yo")
                for mt in range(TT):
                    nc.scalar.activation(yo[:, mt, :], psy[mt][:], COPY)
                nc.sync.dma_start(
                    y_d[th, :, :, nh * 512:(nh + 1) * 512].rearrange(
                        "a p c -> p a c"),
                    yo[:])

    nc.compile()
    return nc


def get_program():
    global _cached_nc
    if _cached_nc is None:
        _cached_nc = _build_program()
    return _cached_nc


def _pack_inputs(hs, wgu, wd):
    """Host-side cast to bf16 + layout packing, per expert."""
    bf16 = ml_dtypes.bfloat16
    in_maps = []
    for e in range(E):
        x = np.ascontiguousarray(hs[e * T:(e + 1) * T].T)          # [H, T]
        xt = x.reshape(KH // 4, 4, P, T).transpose(0, 2, 1, 3)     # [8,128,4,T]
        w = wgu[e].reshape(H, 2, NJ, P)                            # [h,gu,j,c]
        w1 = w.transpose(2, 0, 1, 3).reshape(NJ, KH // KB, KB, P, 2 * P)
        w1 = w1.transpose(0, 1, 3, 2, 4)                           # [j,kb,p,kk,c]
        w2 = wd[e].reshape(KD // KB, KB, P, NH, 512)
        w2 = w2.transpose(3, 0, 2, 1, 4)                           # [nh,kb,p,kk,c]
        in_maps.append({
            "xt": np.ascontiguousarray(xt).astype(bf16),
            "w1": np.ascontiguousarray(w1).astype(bf16),
            "w2": np.ascontiguousarray(w2).astype(bf16),
        })
    return in_maps


def kernel(hidden_states, gate_up_proj, down_proj, run_index=None, _trace=False):
    hs = np.ascontiguousarray(np.asarray(hidden_states, dtype=np.float32))
    wgu = np.ascontiguousarray(np.asarray(gate_up_proj, dtype=np.float32))
    wd = np.ascontiguousarray(np.asarray(down_proj, dtype=np.float32))
    assert hs.shape == (E * T, H) and wgu.shape == (E, H, 2 * D) \
        and wd.shape == (E, D, H)

    nc = get_program()
    in_maps = _pack_inputs(hs, wgu, wd)
    res = run_bass_kernel_spmd(nc, in_maps, core_ids=list(range(E)),
                               trace=_trace)
    out = np.empty((E * T, H), dtype=np.float32)
    for e in range(E):
        out[e * T:(e + 1) * T] = res.results[e]["y"].reshape(T, H)
    if _trace:
        kernel.last_result = res
    return out


# revision 6
# speedup vs baseline: 1.2140x; 1.2140x over previous
"""Llama4 MoE experts (grouped GEMM + SwiGLU) on 8 Trainium2 NeuronCores.

Expert-parallel: core e computes expert e's token block
  Y_e = (silu(X_e @ Wg_e) * (X_e @ Wu_e)) @ Wd_e
with X_e = hidden_states[e*1024:(e+1)*1024]. No collectives needed.

All matmul operands are bf16 (PE full rate, FWL fast weight load); PSUM
accumulation is fp32 and the output is fp32, so end-to-end error stays
~4e-3. The host pre-packs per-core inputs so the device does no casts
and no transposes:
  - xt:  X^T as [8, 128, 4, 1024]  (h on partitions, tokens free)
  - w1:  gate/up interleaved [32, 4, 128, 8, 256] (per d-tile j: 128
         gate cols then 128 up cols, k-batched for 512KB linear DMAs)
  - w2:  Wd as [8, 4, 128, 8, 512] (per 512-wide output col group)
Per-core dataflow:
  MM1: for each d-tile j: accumulate gate/up psum over k (4 PSUM banks
       per group, two groups ping-pong), SwiGLU (Silu on ScalarE + one
       DVE mul) -> act slab bf16 [128, 32, 1024].
  MM2: for each token half/col group: psy[4] accumulate over kd with
       Wd moving; ScalarE evicts to an SBUF row slab, DMA out fp32.
Weights stream once (Wgu) / twice (Wd, once per token half).
"""
from contextlib import ExitStack

import ml_dtypes
import numpy as np

import concourse.bass as bass
import concourse.tile as tile
from concourse import bacc, mybir
from concourse.bass_utils import run_bass_kernel_spmd

P = 128
F32 = mybir.dt.float32
BF16 = mybir.dt.bfloat16
SILU = mybir.ActivationFunctionType.Silu
COPY = mybir.ActivationFunctionType.Copy

E = 8            # experts == cores
T = 1024         # tokens per expert
H = 4096         # hidden
D = 4096         # expert (intermediate) dim

KH = H // P      # 32 contraction tiles for MM1
KD = D // P      # 32 contraction tiles for MM2
NJ = D // P      # 32 gate/up d-tile groups
TC = T // 512    # 2 token chunks (psum free dim 512)
NH = H // 512    # 8 output column groups
TTH = 2          # token halves for MM2
TT = T // TTH // P   # 4 token tiles per half
KB = 8           # k tiles per weight DMA kick

_cached_nc = None


def _build_program():
    nc = bacc.Bacc("TRN2", target_bir_lowering=False, debug=False)
    xt_d = nc.dram_tensor("xt", [KH // 4, P, 4, T], BF16, kind="ExternalInput").ap()
    w1_d = nc.dram_tensor("w1", [NJ, KH // KB, P, KB, 2 * P], BF16,
                          kind="ExternalInput").ap()
    w2_d = nc.dram_tensor("w2", [NH, KD // KB, P, KB, 512], BF16,
                          kind="ExternalInput").ap()
    # y laid out [th, tt, p, h] — linearly identical to [T, H]
    y_d = nc.dram_tensor("y", [TTH, T // TTH // P, P, H], F32,
                         kind="ExternalOutput").ap()

    with tile.TileContext(nc) as tc, ExitStack() as ctx:
        slab = ctx.enter_context(tc.tile_pool(name="slab", bufs=1))
        xt = slab.tile([P, KH, T], BF16, tag="xt")
        act = slab.tile([P, KD, T], BF16, tag="act")

        w1s = ctx.enter_context(tc.tile_pool(name="w1s", bufs=3))
        w2s = ctx.enter_context(tc.tile_pool(name="w2s", bufs=2))
        stmp = ctx.enter_context(tc.tile_pool(name="stmp", bufs=4))
        yout = ctx.enter_context(tc.tile_pool(name="yout", bufs=2))
        ps = ctx.enter_context(tc.tile_pool(name="ps", bufs=8, space="PSUM"))

        # ---- load X^T (8 linear kicks of 1 MiB) ----
        # On the scalar (ACT) HWDGE ring so the first w1 kicks on the sync
        # ring aren't queued behind 8 MiB of xt (rings are FIFO per engine).
        for b in range(KH // 4):
            nc.scalar.dma_start(xt[:, b * 4:(b + 1) * 4, :], xt_d[b])

        # ---- MM1 (gate/up) + SwiGLU -> act ----
        for j in range(NJ):
            psg = [ps.tile([P, 512], F32, tag="ps", name="psg") for _ in range(TC)]
            psu = [ps.tile([P, 512], F32, tag="ps", name="psu") for _ in range(TC)]
            for kb in range(KH // KB):
                wc = w1s.tile([P, KB, 2 * P], BF16, name="w1c")
                nc.sync.dma_start(wc[:], w1_d[j, kb])
                for kk in range(KB):
                    k = kb * KB + kk
                    for which, lst in ((0, psg), (1, psu)):
                        for tc_ in range(TC):
                            nc.tensor.matmul(
                                lst[tc_][:],
                                wc[:, kk, which * P:(which + 1) * P],
                                xt[:, k, tc_ * 512:(tc_ + 1) * 512],
                                start=(k == 0), stop=(k == KH - 1))
            for tc_ in range(TC):
                st = stmp.tile([P, 512], F32, name="st")
                nc.scalar.activation(st[:], psg[tc_][:], SILU)
                nc.vector.tensor_mul(
                    act[:, j, tc_ * 512:(tc_ + 1) * 512], psu[tc_][:], st[:])

        # ---- MM2 (down projection) ----
        for th in range(TTH):
            t0 = th * (T // TTH)
            for nh in range(NH):
                psy = [ps.tile([P, 512], F32, tag="ps", name="psy")
                       for _ in range(TT)]
                for kb in range(KD // KB):
                    wc = w2s.tile([P, KB, 512], BF16, name="w2c")
                    nc.scalar.dma_start(wc[:], w2_d[nh, kb])
                    for kk in range(KB):
                        kd = kb * KB + kk
                        for mt in range(TT):
                            nc.tensor.matmul(
                                psy[mt][:],
                                act[:, kd, t0 + mt * P:t0 + (mt + 1) * P],
                                wc[:, kk, :],
                                start=(kd == 0), stop=(kd == KD - 1))
                yo = yout.tile([P, TT, 512], F32, name="yo")
                for mt in range(TT):
                    nc.scalar.activation(yo[:, mt, :], psy[mt][:], COPY)
                nc.sync.dma_start(
                    y_d[th, :, :, nh * 512:(nh + 1) * 512].rearrange(
                        "a p c -> p a c"),
                    yo[:])

    nc.compile()
    return nc


def get_program():
    global _cached_nc
    if _cached_nc is None:
        _cached_nc = _build_program()
    return _cached_nc


def _pack_inputs(hs, wgu, wd):
    """Host-side cast to bf16 + layout packing, per expert."""
    bf16 = ml_dtypes.bfloat16
    in_maps = []
    for e in range(E):
        x = np.ascontiguousarray(hs[e * T:(e + 1) * T].T)          # [H, T]
        xt = x.reshape(KH // 4, 4, P, T).transpose(0, 2, 1, 3)     # [8,128,4,T]
        w = wgu[e].reshape(H, 2, NJ, P)                            # [h,gu,j,c]
        w1 = w.transpose(2, 0, 1, 3).reshape(NJ, KH // KB, KB, P, 2 * P)
        w1 = w1.transpose(0, 1, 3, 2, 4)                           # [j,kb,p,kk,c]
        w2 = wd[e].reshape(KD // KB, KB, P, NH, 512)
        w2 = w2.transpose(3, 0, 2, 1, 4)                           # [nh,kb,p,kk,c]
        in_maps.append({
            "xt": np.ascontiguousarray(xt).astype(bf16),
            "w1": np.ascontiguousarray(w1).astype(bf16),
            "w2": np.ascontiguousarray(w2).astype(bf16),
        })
    return in_maps


def kernel(hidden_states, gate_up_proj, down_proj, run_index=None, _trace=False):
    hs = np.ascontiguousarray(np.asarray(hidden_states, dtype=np.float32))
    wgu = np.ascontiguousarray(np.asarray(gate_up_proj, dtype=np.float32))
    wd = np.ascontiguousarray(np.asarray(down_proj, dtype=np.float32))
    assert hs.shape == (E * T, H) and wgu.shape == (E, H, 2 * D) \
        and wd.shape == (E, D, H)

    nc = get_program()
    in_maps = _pack_inputs(hs, wgu, wd)
    res = run_bass_kernel_spmd(nc, in_maps, core_ids=list(range(E)),
                               trace=_trace)
    out = np.empty((E * T, H), dtype=np.float32)
    for e in range(E):
        out[e * T:(e + 1) * T] = res.results[e]["y"].reshape(T, H)
    if _trace:
        kernel.last_result = res
    return out


# revision 8
# speedup vs baseline: 1.2231x; 1.0074x over previous
"""Llama4 MoE experts (grouped GEMM + SwiGLU) on 8 Trainium2 NeuronCores.

Expert-parallel: core e computes expert e's token block
  Y_e = (silu(X_e @ Wg_e) * (X_e @ Wu_e)) @ Wd_e
with X_e = hidden_states[e*1024:(e+1)*1024]. No collectives needed.

All matmul operands are bf16 (PE full rate, FWL fast weight load); PSUM
accumulation is fp32 and the output is fp32, so end-to-end error stays
~4e-3. The host pre-packs per-core inputs so the device does no casts
and no transposes:
  - xt:  X^T as [8, 128, 4, 1024]  (h on partitions, tokens free)
  - w1:  gate/up interleaved [32, 4, 128, 8, 256] (per d-tile j: 128
         gate cols then 128 up cols, k-batched for 512KB linear DMAs)
  - w2:  Wd as [8, 4, 128, 8, 512] (per 512-wide output col group)
Per-core dataflow:
  MM1: for each d-tile j: accumulate gate/up psum over k (4 PSUM banks
       per group, two groups ping-pong), SwiGLU (Silu on ScalarE + one
       DVE mul) -> act slab bf16 [128, 32, 1024].
  MM2: for each token half/col group: psy[4] accumulate over kd with
       Wd moving; ScalarE evicts to an SBUF row slab, DMA out fp32.
Weights stream once (Wgu) / twice (Wd, once per token half).
"""
from contextlib import ExitStack

import ml_dtypes
import numpy as np

import concourse.bass as bass
import concourse.tile as tile
from concourse import bacc, mybir
from concourse.bass_utils import run_bass_kernel_spmd

P = 128
F32 = mybir.dt.float32
BF16 = mybir.dt.bfloat16
SILU = mybir.ActivationFunctionType.Silu
COPY = mybir.ActivationFunctionType.Copy

E = 8            # experts == cores
T = 1024         # tokens per expert
H = 4096         # hidden
D = 4096         # expert (intermediate) dim

KH = H // P      # 32 contraction tiles for MM1
KD = D // P      # 32 contraction tiles for MM2
NJ = D // P      # 32 gate/up d-tile groups
TC = T // 512    # 2 token chunks (psum free dim 512)
NH = H // 512    # 8 output column groups
TTH = 2          # token halves for MM2
TT = T // TTH // P   # 4 token tiles per half
KB = 8           # k tiles per weight DMA kick

_cached_nc = None


def _build_program():
    nc = bacc.Bacc("TRN2", target_bir_lowering=False, debug=False)
    xt_d = nc.dram_tensor("xt", [KH // 4, P, 4, T], BF16, kind="ExternalInput").ap()
    w1_d = nc.dram_tensor("w1", [NJ, KH // KB, P, KB, 2 * P], BF16,
                          kind="ExternalInput").ap()
    w2_d = nc.dram_tensor("w2", [NH, KD // KB, P, KB, 512], BF16,
                          kind="ExternalInput").ap()
    # y laid out [th, tt, p, h] — linearly identical to [T, H]
    y_d = nc.dram_tensor("y", [TTH, T // TTH // P, P, H], F32,
                         kind="ExternalOutput").ap()

    with tile.TileContext(nc) as tc, ExitStack() as ctx:
        slab = ctx.enter_context(tc.tile_pool(name="slab", bufs=1))
        xt = slab.tile([P, KH, T], BF16, tag="xt")
        act = slab.tile([P, KD, T], BF16, tag="act")

        w1s = ctx.enter_context(tc.tile_pool(name="w1s", bufs=4))
        w2s = ctx.enter_context(tc.tile_pool(name="w2s", bufs=3))
        stmp = ctx.enter_context(tc.tile_pool(name="stmp", bufs=4))
        yout = ctx.enter_context(tc.tile_pool(name="yout", bufs=4))
        ps = ctx.enter_context(tc.tile_pool(name="ps", bufs=8, space="PSUM"))

        # ---- load X^T (8 linear kicks of 1 MiB) ----
        # On the scalar (ACT) HWDGE ring so the first w1 kicks on the sync
        # ring aren't queued behind 8 MiB of xt (rings are FIFO per engine).
        for b in range(KH // 4):
            nc.scalar.dma_start(xt[:, b * 4:(b + 1) * 4, :], xt_d[b])

        # ---- MM1 (gate/up) + SwiGLU -> act ----
        for j in range(NJ):
            psg = [ps.tile([P, 512], F32, tag="ps", name="psg") for _ in range(TC)]
            psu = [ps.tile([P, 512], F32, tag="ps", name="psu") for _ in range(TC)]
            for kb in range(KH // KB):
                wc = w1s.tile([P, KB, 2 * P], BF16, name="w1c")
                nc.sync.dma_start(wc[:], w1_d[j, kb])
                for kk in range(KB):
                    k = kb * KB + kk
                    for which, lst in ((0, psg), (1, psu)):
                        for tc_ in range(TC):
                            nc.tensor.matmul(
                                lst[tc_][:],
                                wc[:, kk, which * P:(which + 1) * P],
                                xt[:, k, tc_ * 512:(tc_ + 1) * 512],
                                start=(k == 0), stop=(k == KH - 1))
            for tc_ in range(TC):
                st = stmp.tile([P, 512], F32, name="st")
                nc.scalar.activation(st[:], psg[tc_][:], SILU)
                nc.vector.tensor_mul(
                    act[:, j, tc_ * 512:(tc_ + 1) * 512], psu[tc_][:], st[:])

        # ---- MM2 (down projection) ----
        for th in range(TTH):
            t0 = th * (T // TTH)
            for nh in range(NH):
                psy = [ps.tile([P, 512], F32, tag="ps", name="psy")
                       for _ in range(TT)]
                for kb in range(KD // KB):
                    wc = w2s.tile([P, KB, 512], BF16, name="w2c")
                    nc.scalar.dma_start(wc[:], w2_d[nh, kb])
                    for kk in range(KB):
                        kd = kb * KB + kk
                        for mt in range(TT):
                            nc.tensor.matmul(
                                psy[mt][:],
                                act[:, kd, t0 + mt * P:t0 + (mt + 1) * P],
                                wc[:, kk, :],
                                start=(kd == 0), stop=(kd == KD - 1))
                # Evictions split across ScalarE and VectorE so the four
                # PSUM banks recycle ~2x faster at nh boundaries; per-tile
                # DMA lets the tail drain as soon as each copy lands.
                for mt in range(TT):
                    yo = yout.tile([P, 512], F32, name="yo")
                    if mt % 2 == 0:
                        nc.scalar.activation(yo[:], psy[mt][:], COPY)
                    else:
                        nc.vector.tensor_copy(yo[:], psy[mt][:])
                    nc.sync.dma_start(
                        y_d[th, mt, :, nh * 512:(nh + 1) * 512], yo[:])

    nc.compile()
    return nc


def get_program():
    global _cached_nc
    if _cached_nc is None:
        _cached_nc = _build_program()
    return _cached_nc


def _pack_inputs(hs, wgu, wd):
    """Host-side cast to bf16 + layout packing, per expert."""
    bf16 = ml_dtypes.bfloat16
    in_maps = []
    for e in range(E):
        x = np.ascontiguousarray(hs[e * T:(e + 1) * T].T)          # [H, T]
        xt = x.reshape(KH // 4, 4, P, T).transpose(0, 2, 1, 3)     # [8,128,4,T]
        w = wgu[e].reshape(H, 2, NJ, P)                            # [h,gu,j,c]
        w1 = w.transpose(2, 0, 1, 3).reshape(NJ, KH // KB, KB, P, 2 * P)
        w1 = w1.transpose(0, 1, 3, 2, 4)                           # [j,kb,p,kk,c]
        w2 = wd[e].reshape(KD // KB, KB, P, NH, 512)
        w2 = w2.transpose(3, 0, 2, 1, 4)                           # [nh,kb,p,kk,c]
        in_maps.append({
            "xt": np.ascontiguousarray(xt).astype(bf16),
            "w1": np.ascontiguousarray(w1).astype(bf16),
            "w2": np.ascontiguousarray(w2).astype(bf16),
        })
    return in_maps


def kernel(hidden_states, gate_up_proj, down_proj, run_index=None, _trace=False):
    hs = np.ascontiguousarray(np.asarray(hidden_states, dtype=np.float32))
    wgu = np.ascontiguousarray(np.asarray(gate_up_proj, dtype=np.float32))
    wd = np.ascontiguousarray(np.asarray(down_proj, dtype=np.float32))
    assert hs.shape == (E * T, H) and wgu.shape == (E, H, 2 * D) \
        and wd.shape == (E, D, H)

    nc = get_program()
    in_maps = _pack_inputs(hs, wgu, wd)
    res = run_bass_kernel_spmd(nc, in_maps, core_ids=list(range(E)),
                               trace=_trace)
    out = np.empty((E * T, H), dtype=np.float32)
    for e in range(E):
        out[e * T:(e + 1) * T] = res.results[e]["y"].reshape(T, H)
    if _trace:
        kernel.last_result = res
    return out
